# revision 47
# baseline (speedup 1.0000x reference)
"""Trainium2 Bass kernel for nn_LocalDiscriminator (patch-GAN style loss).

Reference computation (full shapes):
    x: [32, 1024, 64, 64] f32, w: [1, 1024] f32, b: [1] f32, mode: scalar int
    logits = einsum('bchw,c->bhw', x, w[0]) + b[0]
    z = sigmoid(logits)
    loss = mean(softplus(z) - z * mode)        # scalar f32

Strategy: data-parallel over the batch dim — 4 batches per core on 8 cores.
Each core streams its 64 MiB shard of x through the TensorEngine: the
channel contraction uses lhsT = [w_col, w_col] ([128, 2] stationary, f32r
so the moving data streams at 1 cycle/row instead of fp32's 4), writing
IDENTICAL logits to two PSUM partitions. One ScalarEngine tanh per group —
with per-partition scale/bias APs — evaluates both reductions at once, and
its accum_out port emits the per-group sums for free (one ACT LUT table for
the whole kernel; the per-group sums go straight to DRAM and the final
reduction happens on the host):
    partition 0:  sum tanh(FA*t + FA*b+FB)   -> softplus fit
    partition 1:  sum tanh(0.5*t + 0.5*b)    -> exact sigmoid identity
where t is the raw logit. Host combination:
    sum(z)            = N/2 + S_z/2                             (exact)
    sum(softplus(z)) ~= N*FC0 + FC1*S_f                         (fitted)
    loss = (sum(softplus(z)) - mode*sum(z)) / N
The fit softplus(sigmoid(t)) ~= FC0 + FC1*tanh(FA*t+FB) has max |err|
9.8e-4 per element on t in [-4.5, 4.5] and its mean error over the
N(0, ~0.64) logit distribution cancels to ~1e-7 — the loss is a mean of
131072 such elements, so even the worst-case systematic error (1.4e-3,
saturated logits) is 14x inside the 2e-2 gate.

The kernel is HBM-bandwidth-bound: ~64 MiB/core at 360 GB/s (~186.4 us of
pure transfer). Things that keep the end-to-end time near that floor
(~192.5 us total vs ~200.5 us before):
  * Channel fold c = 8q + i (chunk i puts channel 8q+i on partition q):
    the w scatter then has 32 B descriptors (56 ns of DMA time) instead of
    4 B ones (448 ns), and each x pair-load still moves two ADJACENT
    channel rows per partition = 32 KiB contiguous descriptors.
  * The last batch streams COLUMN-major in three waves (cols 0:2048,
    2048:3584, 3584:4096 = 4+3+1 psum banks), each wave's ACT running
    while later columns are still in flight. The final 512 cols split
    into two 256-col halves in SEPARATE psum banks (no false WAR), so
    the work gated on the very last DMA piece (128 KiB: 256 cols of
    chunk 7) is one [2, 256] matmul plus one 256-col no-accum ACT whose
    tanh vector lands next to the group sums — a single ~1 KiB result
    DMA ships everything. Tail past the last x byte: ~4.7 us (was ~12).
  * Bass.__init__'s const-tile memsets + entry barrier are skipped (the
    consts are unused here), so the first x transfer dispatches at
    ~1.35 us instead of ~1.97 us.
"""

import os
import sys

import numpy as np

_REPO_CANDIDATES = ("/opt/trn_rl_repo", "/root/.axon_site/_ro/trn_rl_repo")
for _p in _REPO_CANDIDATES:
    if os.path.isdir(_p) and _p not in sys.path:
        sys.path.insert(0, _p)

import concourse.bacc as bacc
import concourse.bass as bass
import concourse.mybir as mybir
import concourse.tile as tile
from concourse.bass_utils import run_bass_kernel_spmd

N_CORES = 8
B_FULL, C, H, W = 32, 1024, 64, 64
B_LOCAL = B_FULL // N_CORES          # 4 batches per core
HW = H * W                           # 4096 spatial positions per batch
C_CHUNKS = C // 128                  # 8 chunks of 128 channels
N_GROUPS = (B_LOCAL - 1) * 2 + 3     # accum act-groups (2/batch + 3 last)
TAILV = 256                          # tail cols shipped as raw tanh values
SUMW = N_GROUPS + TAILV              # width of the result row

# softplus(sigmoid(t)) ~= FC0 + FC1 * tanh(FA*t + FB)
FC0 = 1.0028824947566075
FC1 = 0.30899789558232016
FA = 0.5078652298016119
FB = -0.09351045988102749

F32 = mybir.dt.float32
F32R = mybir.dt.float32r

_nc_cache = None
_exec_cache = None

# Bass.__init__ unconditionally emits four Pool-engine const memsets
# (const-f32-0.0/1.0, const-bf16-1.0, const-u8-127) plus an all-engine
# barrier. The memsets serialize on the Pool SEQ (95 ns Q7 launch each),
# so the barrier — and with it the first x DMA dispatch — completes only
# at ~616 ns instead of ~100 ns. This kernel never touches the const APs
# (every activation passes explicit scale/bias tiles), so both are dead
# weight: skip them during construction only. TileContext's own exit
# drain/barrier/sem-clear sequence is emitted outside this scope and is
# untouched.
_IN_BASS_INIT = False
_ORIG_MEMSET = bass.BassSharedVectorInterface.memset
_ORIG_BARRIER = bass.Bass.all_engine_barrier


def _patched_memset(self, ap, constant):
    if _IN_BASS_INIT:
        return None
    return _ORIG_MEMSET(self, ap, constant)


def _patched_barrier(self, *, sem_only=False):
    if _IN_BASS_INIT:
        return None
    return _ORIG_BARRIER(self, sem_only=sem_only)


bass.BassSharedVectorInterface.memset = _patched_memset
bass.BassGpSimd.memset = _patched_memset
bass.BassVectorEngine.memset = _patched_memset
bass.Bass.all_engine_barrier = _patched_barrier


def _build_nc():
    global _IN_BASS_INIT
    _IN_BASS_INIT = True
    try:
        nc = bacc.Bacc("TRN2", target_bir_lowering=False, debug=False,
                       num_devices=N_CORES)
    finally:
        _IN_BASS_INIT = False

    x = nc.dram_tensor("x", [B_LOCAL, C, H, W], F32, kind="ExternalInput").ap()
    w = nc.dram_tensor("w", [1, C], F32, kind="ExternalInput").ap()
    # aff[p] = (scale, bias) for the tanh on psum partition p; computed on
    # the host from the Linear bias b:
    #   row 0 = (FA, FA*b+FB)   (softplus fit), row 1 = (0.5, 0.5*b) (sigmoid)
    aff = nc.dram_tensor("aff", [2, 2], F32, kind="ExternalInput").ap()
    # Row layout: cols [0, N_GROUPS) hold per-group tanh SUMS (ACT accum
    # port); cols [N_GROUPS, SUMW) hold the last 256 columns' RAW tanh
    # values (a no-accum ACT is cheaper on the critical tail than an
    # accum one over 512 cols). The host just sums each row, so both
    # kinds of entry combine identically.
    out = nc.dram_tensor("out", [2, SUMW], F32,
                         kind="ExternalOutput").ap()

    # Channel fold: chunk i holds channels {8q + i}, so partition q of a
    # chunk-pair tile reads two ADJACENT 16 KiB channel rows (32 KiB
    # contiguous descriptors) and w folds to [128, 8] with 32 B descriptors.
    xq = x.rearrange("b (q t) h w -> b q t (h w)", t=C_CHUNKS)

    with tile.TileContext(nc) as tc:
        with (
            tc.tile_pool(name="xpool", bufs=6) as xpool,
            tc.tile_pool(name="const", bufs=1) as cpool,
            tc.tile_pool(name="sums", bufs=1) as spool,
            tc.tile_pool(name="dump", bufs=1) as dpool,
            tc.tile_pool(name="psum", bufs=2, space="PSUM") as pspool,
        ):
            # Two copies of w side by side: lhsT [128, 2] per chunk makes the
            # matmul write identical logits to TWO psum partitions, so one
            # ACT tanh with per-partition scale/bias evaluates both the
            # softplus fit (partition 0) and the exact sigmoid identity
            # (partition 1) in a single instruction.
            # w2[q, k, i] = w[0, 8*q + i] for k in {0,1}.
            w2 = cpool.tile([128, 2, C_CHUNKS], F32R, tag="w")
            nc.gpsimd.dma_start(
                out=w2[:, 0, :],
                in_=w[0].bitcast(F32R).rearrange("(p i) -> p i", p=128))
            # replicate the second stationary copy on the idle VectorE
            # instead of paying a second scattered DMA on the shared engines
            nc.vector.tensor_copy(w2[:, 1, :], w2[:, 0, :])
            aff_t = cpool.tile([2, 2], F32, tag="aff")
            nc.gpsimd.dma_start(out=aff_t[:], in_=aff[:])

            # sums[0, i] = sum tanh(FA*t+FB') of group i  (softplus fit)
            # sums[1, i] = sum tanh(t/2+b/2) of group i   (sigmoid)
            sums = spool.tile([2, SUMW], F32, tag="sums")

            def emit_act(ps, nbank, ncols, idx):
                # Only the accum_out sums are consumed; the elementwise
                # tanh output goes to a scratch tile.
                dump = dpool.tile([2, 2048], F32, tag="dump")
                nc.scalar.activation(
                    dump[:2, :ncols],
                    ps[0:2, 0:nbank, :].rearrange("p a b -> p (a b)"),
                    mybir.ActivationFunctionType.Tanh,
                    bias=aff_t[:, 1:2], scale=aff_t[:, 0:1],
                    accum_out=sums[0:2, idx:idx + 1],
                )

            def emit_mm(ps, jj, rhs, c, ncols=512, colofs=0):
                nc.tensor.matmul(
                    ps[0:2, jj, colofs:colofs + ncols],
                    lhsT=w2[:, :, c],
                    rhs=rhs,
                    start=(c == 0),
                    stop=(c == C_CHUNKS - 1),
                )

            # Batches 0..B_LOCAL-2: stream chunk-pair-major (4 MiB loads),
            # two 2048-col groups per batch on all 8 psum banks.
            for bi in range(B_LOCAL - 1):
                tiles = []
                for p in range(C_CHUNKS // 2):
                    xt = xpool.tile([128, 2, HW], F32R, tag="x",
                                    name=f"xt_{bi}_{p}")
                    nc.sync.dma_start(
                        out=xt[:],
                        in_=xq[bi, :, 2 * p:2 * p + 2, :].bitcast(F32R))
                    tiles.append(xt)
                for gi, tg in enumerate(("t4a", "t4b")):
                    ps_g = pspool.tile([2, 4, 512], F32,
                                       name=f"ps_{bi}_{gi}", tag=tg, bufs=1)
                    for c in range(C_CHUNKS):
                        xt = tiles[c // 2]
                        for jj in range(4):
                            col = gi * 2048 + jj * 512
                            emit_mm(ps_g, jj, xt[:, c % 2, col:col + 512], c)
                    emit_act(ps_g, 4, 2048, bi * 2 + gi)

            # Last batch: stream COLUMN-major in three waves so each act
            # group completes (and its ACT runs) while later columns are
            # still in flight. The final wave's 512 cols split into two
            # 256-col halves: the first gets a normal accum ACT (early,
            # off the critical path), the second a NO-accum ACT whose
            # tanh vector lands in the sums tile — the very last DMA
            # piece gates only one [2, 256] matmul and that short ACT.
            bi = B_LOCAL - 1
            tiles = [xpool.tile([128, 2, HW], F32R, tag="x",
                                name=f"xt_last_{p}")
                     for p in range(C_CHUNKS // 2)]
            # (col0, ncols, psum tag, group index). The tail wave reuses
            # t4a: its banks are free once g0's ACT has read them.
            for c0, ncols, tg, idx in ((0, 2048, "t4a", 6),
                                       (2048, 1536, "t4b", 7)):
                nbank = ncols // 512
                ps_g = pspool.tile([2, 4, 512], F32,
                                   name=f"ps_last_{c0}", tag=tg, bufs=1)
                for p in range(C_CHUNKS // 2):
                    xt = tiles[p]
                    nc.sync.dma_start(
                        out=xt[:, :, c0:c0 + ncols],
                        in_=xq[bi, :, 2 * p:2 * p + 2,
                               c0:c0 + ncols].bitcast(F32R))
                    for h in range(2):
                        for jj in range(nbank):
                            col = c0 + jj * 512
                            emit_mm(ps_g, jj,
                                    xt[:, h, col:col + 512], 2 * p + h)
                emit_act(ps_g, nbank, ncols, idx)

            # Tail wave (cols 3584:4096): the two 256-col halves live in
            # SEPARATE psum banks so the second half's matmuls don't
            # falsely serialize behind the first half's ACT.
            c0 = 3584
            ps_g = pspool.tile([2, 4, 512], F32,
                               name="ps_last_tail", tag="t4a", bufs=1)
            for p in range(C_CHUNKS // 2 - 1):
                xt = tiles[p]
                nc.sync.dma_start(
                    out=xt[:, :, c0:c0 + 512],
                    in_=xq[bi, :, 2 * p:2 * p + 2,
                           c0:c0 + 512].bitcast(F32R))
                for h in range(2):
                    for half in range(2):
                        lo = c0 + 256 * half
                        emit_mm(ps_g, half, xt[:, h, lo:lo + 256],
                                2 * p + h, ncols=256)
            # Final pair (chunks 6, 7): four 128 KiB pieces, (c6, c7) per
            # half, so each half's accumulation closes as early as
            # possible and the last piece gates one [2, 256] matmul.
            xt = tiles[C_CHUNKS // 2 - 1]
            for half in range(2):
                lo = c0 + 256 * half
                for c in (6, 7):
                    nc.sync.dma_start(
                        out=xt[:, c % 2, lo:lo + 256],
                        in_=xq[bi, :, c, lo:lo + 256].bitcast(F32R))
                    emit_mm(ps_g, half, xt[:, c % 2, lo:lo + 256],
                            c, ncols=256)
                if half == 0:
                    dump = dpool.tile([2, 2048], F32, tag="dump",
                                      name="dump_tail")
                    nc.scalar.activation(
                        dump[:2, :256],
                        ps_g[0:2, 0, 0:256],
                        mybir.ActivationFunctionType.Tanh,
                        bias=aff_t[:, 1:2], scale=aff_t[:, 0:1],
                        accum_out=sums[0:2, 8:9],
                    )
                else:
                    nc.scalar.activation(
                        sums[0:2, N_GROUPS:SUMW],
                        ps_g[0:2, 1, 0:256],
                        mybir.ActivationFunctionType.Tanh,
                        bias=aff_t[:, 1:2], scale=aff_t[:, 0:1],
                    )

            nc.sync.dma_start(out=out[:], in_=sums[:])

    nc.compile()
    return nc


def _get_nc():
    global _nc_cache
    if _nc_cache is None:
        _nc_cache = _build_nc()
    return _nc_cache


def _get_exec():
    """Compile the 8-core SPMD executable once and cache the jitted callable
    (run_bass_kernel_spmd rebuilds + recompiles the NEFF on every call)."""
    global _exec_cache
    if _exec_cache is not None:
        return _exec_cache

    import jax
    import concourse.mybir as _mybir
    from concourse import bass2jax
    from jax.experimental.shard_map import shard_map
    from jax.sharding import Mesh, PartitionSpec

    nc = _get_nc()
    bass2jax.install_neuronx_cc_hook()

    partition_name = (nc.partition_id_tensor.name
                      if nc.partition_id_tensor else None)
    in_names, out_names, out_avals = [], [], []
    for alloc in nc.m.functions[0].allocations:
        if not isinstance(alloc, _mybir.MemoryLocationSet):
            continue
        name = alloc.memorylocations[0].name
        if alloc.kind == "ExternalInput":
            if name != partition_name:
                in_names.append(name)
        elif alloc.kind == "ExternalOutput":
            shape = tuple(alloc.tensor_shape)
            dtype = _mybir.dt.np(alloc.dtype)
            out_names.append(name)
            out_avals.append(jax.core.ShapedArray(shape, dtype))
    n_params = len(in_names)
    all_in_names = list(in_names) + list(out_names)
    if partition_name is not None:
        all_in_names.append(partition_name)

    def _body(*args):
        operands = list(args)
        if partition_name is not None:
            operands.append(bass2jax.partition_id_tensor())
        outs = bass2jax._bass_exec_p.bind(
            *operands,
            out_avals=tuple(out_avals),
            in_names=tuple(all_in_names),
            out_names=tuple(out_names),
            lowering_input_output_aliases=(),
            sim_require_finite=True,
            sim_require_nnan=True,
            nc=nc,
        )
        return tuple(outs)

    devices = jax.devices()[:N_CORES]
    mesh = Mesh(np.asarray(devices), ("core",))
    n_outs = len(out_names)
    sharded = jax.jit(
        shard_map(
            _body, mesh=mesh,
            in_specs=(PartitionSpec("core"),) * (n_params + n_outs),
            out_specs=(PartitionSpec("core"),) * n_outs,
            check_rep=False,
        ),
        donate_argnums=tuple(range(n_params, n_params + n_outs)),
        keep_unused=True,
    )
    _exec_cache = (sharded, in_names, out_names, out_avals)
    return _exec_cache


def _run_spmd(in_maps):
    """Run the cached executable; returns list of per-core output dicts."""
    sharded, in_names, out_names, out_avals = _get_exec()
    concat_in = [
        np.concatenate([np.asarray(m[name]) for m in in_maps], axis=0)
        for name in in_names
    ]
    concat_zeros = [
        np.zeros((N_CORES * av.shape[0], *av.shape[1:]), av.dtype)
        for av in out_avals
    ]
    out_arrs = sharded(*concat_in, *concat_zeros)
    return [
        {name: np.asarray(out_arrs[i]).reshape(N_CORES, *out_avals[i].shape)[c]
         for i, name in enumerate(out_names)}
        for c in range(N_CORES)
    ]


def kernel(x: np.ndarray, w: np.ndarray, b: np.ndarray, mode) -> np.ndarray:
    x = np.ascontiguousarray(np.asarray(x, dtype=np.float32))
    w = np.ascontiguousarray(np.asarray(w, dtype=np.float32))
    b = np.ascontiguousarray(np.asarray(b, dtype=np.float32))
    assert x.shape == (B_FULL, C, H, W), x.shape

    b0 = float(b.reshape(-1)[0])
    aff = np.array([[FA, FA * b0 + FB], [0.5, 0.5 * b0]], dtype=np.float32)
    in_maps = [
        {"x": x[i * B_LOCAL:(i + 1) * B_LOCAL], "w": w, "aff": aff}
        for i in range(N_CORES)
    ]
    try:
        results = _run_spmd(in_maps)
    except Exception:
        nc = _get_nc()
        results = run_bass_kernel_spmd(nc, in_maps, list(range(N_CORES))).results
    partial = np.stack([r["out"] for r in results])  # [8, 2, SUMW]

    n_total = float(B_FULL * HW)
    # Row 0/1 mix per-group SUMS and the tail's raw tanh VALUES — both
    # just add.
    sum_f = float(partial[:, 0, :].sum())
    sum_z = float(partial[:, 1, :].sum())
    s_sp = n_total * FC0 + FC1 * sum_f
    s_z = n_total / 2.0 + sum_z / 2.0
    y = float(np.asarray(mode))
    loss = (s_sp - y * s_z) / n_total
    return np.float32(loss)


# revision 48
# speedup vs baseline: 1.0014x; 1.0014x over previous
"""Trainium2 Bass kernel for nn_LocalDiscriminator (patch-GAN style loss).

Reference computation (full shapes):
    x: [32, 1024, 64, 64] f32, w: [1, 1024] f32, b: [1] f32, mode: scalar int
    logits = einsum('bchw,c->bhw', x, w[0]) + b[0]
    z = sigmoid(logits)
    loss = mean(softplus(z) - z * mode)        # scalar f32

Strategy: data-parallel over the batch dim — 4 batches per core on 8 cores.
Each core streams its 64 MiB shard of x through the TensorEngine: the
channel contraction uses lhsT = [w_col, w_col] ([128, 2] stationary, f32r
so the moving data streams at 1 cycle/row instead of fp32's 4), writing
IDENTICAL logits to two PSUM partitions. One ScalarEngine tanh per group —
with per-partition scale/bias APs — evaluates both reductions at once, and
its accum_out port emits the per-group sums for free (one ACT LUT table for
the whole kernel; the per-group sums go straight to DRAM and the final
reduction happens on the host):
    partition 0:  sum tanh(FA*t + FA*b+FB)   -> softplus fit
    partition 1:  sum tanh(0.5*t + 0.5*b)    -> exact sigmoid identity
where t is the raw logit. Host combination:
    sum(z)            = N/2 + S_z/2                             (exact)
    sum(softplus(z)) ~= N*FC0 + FC1*S_f                         (fitted)
    loss = (sum(softplus(z)) - mode*sum(z)) / N
The fit softplus(sigmoid(t)) ~= FC0 + FC1*tanh(FA*t+FB) has max |err|
9.8e-4 per element on t in [-4.5, 4.5] and its mean error over the
N(0, ~0.64) logit distribution cancels to ~1e-7 — the loss is a mean of
131072 such elements, so even the worst-case systematic error (1.4e-3,
saturated logits) is 14x inside the 2e-2 gate.

The kernel is HBM-bandwidth-bound: ~64 MiB/core at 360 GB/s (~186.4 us of
pure transfer). Things that keep the end-to-end time near that floor
(~192.5 us total vs ~200.5 us before):
  * Channel fold c = 8q + i (chunk i puts channel 8q+i on partition q):
    the w scatter then has 32 B descriptors (56 ns of DMA time) instead of
    4 B ones (448 ns), and each x pair-load still moves two ADJACENT
    channel rows per partition = 32 KiB contiguous descriptors.
  * The last batch streams COLUMN-major in three waves (cols 0:2048,
    2048:3584, 3584:4096 = 4+3+1 psum banks), each wave's ACT running
    while later columns are still in flight. The final 512 cols split
    into two 256-col halves in SEPARATE psum banks (no false WAR), so
    the work gated on the very last DMA piece (128 KiB: 256 cols of
    chunk 7) is one [2, 256] matmul plus one 256-col no-accum ACT whose
    tanh vector lands next to the group sums — a single ~1 KiB result
    DMA ships everything. Tail past the last x byte: ~4.7 us (was ~12).
  * Bass.__init__'s const-tile memsets + entry barrier are skipped (the
    consts are unused here), so the first x transfer dispatches at
    ~1.35 us instead of ~1.97 us.
"""

import os
import sys

import numpy as np

_REPO_CANDIDATES = ("/opt/trn_rl_repo", "/root/.axon_site/_ro/trn_rl_repo")
for _p in _REPO_CANDIDATES:
    if os.path.isdir(_p) and _p not in sys.path:
        sys.path.insert(0, _p)

import concourse.bacc as bacc
import concourse.bass as bass
import concourse.mybir as mybir
import concourse.tile as tile
from concourse.bass_utils import run_bass_kernel_spmd

N_CORES = 8
B_FULL, C, H, W = 32, 1024, 64, 64
B_LOCAL = B_FULL // N_CORES          # 4 batches per core
HW = H * W                           # 4096 spatial positions per batch
C_CHUNKS = C // 128                  # 8 chunks of 128 channels
N_GROUPS = (B_LOCAL - 1) * 2 + 3     # accum act-groups (2/batch + 3 last)
TAILV = 256                          # tail cols shipped as raw tanh values
SUMW = N_GROUPS + TAILV              # width of the result row

# softplus(sigmoid(t)) ~= FC0 + FC1 * tanh(FA*t + FB)
FC0 = 1.0028824947566075
FC1 = 0.30899789558232016
FA = 0.5078652298016119
FB = -0.09351045988102749

F32 = mybir.dt.float32
F32R = mybir.dt.float32r

_nc_cache = None
_exec_cache = None

# Bass.__init__ unconditionally emits four Pool-engine const memsets
# (const-f32-0.0/1.0, const-bf16-1.0, const-u8-127) plus an all-engine
# barrier. The memsets serialize on the Pool SEQ (95 ns Q7 launch each),
# so the barrier — and with it the first x DMA dispatch — completes only
# at ~616 ns instead of ~100 ns. This kernel never touches the const APs
# (every activation passes explicit scale/bias tiles), so both are dead
# weight: skip them during construction only. TileContext's own exit
# drain/barrier/sem-clear sequence is emitted outside this scope and is
# untouched.
_IN_BASS_INIT = False
_ORIG_MEMSET = bass.BassSharedVectorInterface.memset
_ORIG_BARRIER = bass.Bass.all_engine_barrier


def _patched_memset(self, ap, constant):
    if _IN_BASS_INIT:
        return None
    return _ORIG_MEMSET(self, ap, constant)


def _patched_barrier(self, *, sem_only=False):
    if _IN_BASS_INIT:
        return None
    return _ORIG_BARRIER(self, sem_only=sem_only)


bass.BassSharedVectorInterface.memset = _patched_memset
bass.BassGpSimd.memset = _patched_memset
bass.BassVectorEngine.memset = _patched_memset
bass.Bass.all_engine_barrier = _patched_barrier


# TileContext's exit emits drain -> barrier -> semaphore range-clear ->
# barrier. The final barrier only guards the range-clear against engines
# racing ahead WITHIN this launch — but nothing follows it, and between
# launches the runtime itself serializes (each execution starts after the
# previous one fully completed). Skipping it shaves the last ~250 ns of
# the kernel tail. The drain, first barrier, and the clear itself are
# kept intact.
from concourse.vector_clock import ScopedClock as _ScopedClock


def _patched_drain_and_barrier(self, tick_clock, wait_clock):
    drain_inst = self.nc.sync.drain()
    wait_clock.add_sem_waits(
        drain_inst.ins, _ScopedClock({None: tick_clock.global_clock})
    )
    self.nc.all_engine_barrier()
    popped = self.nc._tile_sem_poison_stack.pop()
    assert popped is self._sem_poison
    self.nc.clear_and_free_semaphores(list(self.sems.allocated().values()))


tile.TileContext._drain_and_barrier = _patched_drain_and_barrier


def _build_nc():
    global _IN_BASS_INIT
    _IN_BASS_INIT = True
    try:
        nc = bacc.Bacc("TRN2", target_bir_lowering=False, debug=False,
                       num_devices=N_CORES)
    finally:
        _IN_BASS_INIT = False

    x = nc.dram_tensor("x", [B_LOCAL, C, H, W], F32, kind="ExternalInput").ap()
    w = nc.dram_tensor("w", [1, C], F32, kind="ExternalInput").ap()
    # aff[p] = (scale, bias) for the tanh on psum partition p; computed on
    # the host from the Linear bias b:
    #   row 0 = (FA, FA*b+FB)   (softplus fit), row 1 = (0.5, 0.5*b) (sigmoid)
    aff = nc.dram_tensor("aff", [2, 2], F32, kind="ExternalInput").ap()
    # Row layout: cols [0, N_GROUPS) hold per-group tanh SUMS (ACT accum
    # port); cols [N_GROUPS, SUMW) hold the last 256 columns' RAW tanh
    # values (a no-accum ACT is cheaper on the critical tail than an
    # accum one over 512 cols). The host just sums each row, so both
    # kinds of entry combine identically.
    out = nc.dram_tensor("out", [2, SUMW], F32,
                         kind="ExternalOutput").ap()

    # Channel fold: chunk i holds channels {8q + i}, so partition q of a
    # chunk-pair tile reads two ADJACENT 16 KiB channel rows (32 KiB
    # contiguous descriptors) and w folds to [128, 8] with 32 B descriptors.
    xq = x.rearrange("b (q t) h w -> b q t (h w)", t=C_CHUNKS)

    with tile.TileContext(nc) as tc:
        with (
            tc.tile_pool(name="xpool", bufs=6) as xpool,
            tc.tile_pool(name="const", bufs=1) as cpool,
            tc.tile_pool(name="sums", bufs=1) as spool,
            tc.tile_pool(name="dump", bufs=1) as dpool,
            tc.tile_pool(name="psum", bufs=2, space="PSUM") as pspool,
        ):
            # Two copies of w side by side: lhsT [128, 2] per chunk makes the
            # matmul write identical logits to TWO psum partitions, so one
            # ACT tanh with per-partition scale/bias evaluates both the
            # softplus fit (partition 0) and the exact sigmoid identity
            # (partition 1) in a single instruction.
            # w2[q, k, i] = w[0, 8*q + i] for k in {0,1}.
            w2 = cpool.tile([128, 2, C_CHUNKS], F32R, tag="w")
            nc.gpsimd.dma_start(
                out=w2[:, 0, :],
                in_=w[0].bitcast(F32R).rearrange("(p i) -> p i", p=128))
            # replicate the second stationary copy on the idle VectorE
            # instead of paying a second scattered DMA on the shared engines
            nc.vector.tensor_copy(w2[:, 1, :], w2[:, 0, :])
            aff_t = cpool.tile([2, 2], F32, tag="aff")
            nc.gpsimd.dma_start(out=aff_t[:], in_=aff[:])

            # sums[0, i] = sum tanh(FA*t+FB') of group i  (softplus fit)
            # sums[1, i] = sum tanh(t/2+b/2) of group i   (sigmoid)
            sums = spool.tile([2, SUMW], F32, tag="sums")

            def emit_act(ps, nbank, ncols, idx):
                # Only the accum_out sums are consumed; the elementwise
                # tanh output goes to a scratch tile.
                dump = dpool.tile([2, 2048], F32, tag="dump")
                nc.scalar.activation(
                    dump[:2, :ncols],
                    ps[0:2, 0:nbank, :].rearrange("p a b -> p (a b)"),
                    mybir.ActivationFunctionType.Tanh,
                    bias=aff_t[:, 1:2], scale=aff_t[:, 0:1],
                    accum_out=sums[0:2, idx:idx + 1],
                )

            def emit_mm(ps, jj, rhs, c, ncols=512, colofs=0):
                nc.tensor.matmul(
                    ps[0:2, jj, colofs:colofs + ncols],
                    lhsT=w2[:, :, c],
                    rhs=rhs,
                    start=(c == 0),
                    stop=(c == C_CHUNKS - 1),
                )

            # Batches 0..B_LOCAL-2: stream chunk-pair-major (4 MiB loads),
            # two 2048-col groups per batch on all 8 psum banks.
            for bi in range(B_LOCAL - 1):
                tiles = []
                for p in range(C_CHUNKS // 2):
                    xt = xpool.tile([128, 2, HW], F32R, tag="x",
                                    name=f"xt_{bi}_{p}")
                    nc.sync.dma_start(
                        out=xt[:],
                        in_=xq[bi, :, 2 * p:2 * p + 2, :].bitcast(F32R))
                    tiles.append(xt)
                for gi, tg in enumerate(("t4a", "t4b")):
                    ps_g = pspool.tile([2, 4, 512], F32,
                                       name=f"ps_{bi}_{gi}", tag=tg, bufs=1)
                    for c in range(C_CHUNKS):
                        xt = tiles[c // 2]
                        for jj in range(4):
                            col = gi * 2048 + jj * 512
                            emit_mm(ps_g, jj, xt[:, c % 2, col:col + 512], c)
                    emit_act(ps_g, 4, 2048, bi * 2 + gi)

            # Last batch: stream COLUMN-major in three waves so each act
            # group completes (and its ACT runs) while later columns are
            # still in flight. The final wave's 512 cols split into two
            # 256-col halves: the first gets a normal accum ACT (early,
            # off the critical path), the second a NO-accum ACT whose
            # tanh vector lands in the sums tile — the very last DMA
            # piece gates only one [2, 256] matmul and that short ACT.
            bi = B_LOCAL - 1
            tiles = [xpool.tile([128, 2, HW], F32R, tag="x",
                                name=f"xt_last_{p}")
                     for p in range(C_CHUNKS // 2)]
            # (col0, ncols, psum tag, group index). The tail wave reuses
            # t4a: its banks are free once g0's ACT has read them.
            for c0, ncols, tg, idx in ((0, 2048, "t4a", 6),
                                       (2048, 1536, "t4b", 7)):
                nbank = ncols // 512
                ps_g = pspool.tile([2, 4, 512], F32,
                                   name=f"ps_last_{c0}", tag=tg, bufs=1)
                for p in range(C_CHUNKS // 2):
                    xt = tiles[p]
                    nc.sync.dma_start(
                        out=xt[:, :, c0:c0 + ncols],
                        in_=xq[bi, :, 2 * p:2 * p + 2,
                               c0:c0 + ncols].bitcast(F32R))
                    for h in range(2):
                        for jj in range(nbank):
                            col = c0 + jj * 512
                            emit_mm(ps_g, jj,
                                    xt[:, h, col:col + 512], 2 * p + h)
                emit_act(ps_g, nbank, ncols, idx)

            # Tail wave (cols 3584:4096): the two 256-col halves live in
            # SEPARATE psum banks so the second half's matmuls don't
            # falsely serialize behind the first half's ACT.
            c0 = 3584
            ps_g = pspool.tile([2, 4, 512], F32,
                               name="ps_last_tail", tag="t4a", bufs=1)
            for p in range(C_CHUNKS // 2 - 1):
                xt = tiles[p]
                nc.sync.dma_start(
                    out=xt[:, :, c0:c0 + 512],
                    in_=xq[bi, :, 2 * p:2 * p + 2,
                           c0:c0 + 512].bitcast(F32R))
                for h in range(2):
                    for half in range(2):
                        lo = c0 + 256 * half
                        emit_mm(ps_g, half, xt[:, h, lo:lo + 256],
                                2 * p + h, ncols=256)
            # Final pair (chunks 6, 7): four 128 KiB pieces, (c6, c7) per
            # half, so each half's accumulation closes as early as
            # possible and the last piece gates one [2, 256] matmul.
            xt = tiles[C_CHUNKS // 2 - 1]
            for half in range(2):
                lo = c0 + 256 * half
                for c in (6, 7):
                    nc.sync.dma_start(
                        out=xt[:, c % 2, lo:lo + 256],
                        in_=xq[bi, :, c, lo:lo + 256].bitcast(F32R))
                    emit_mm(ps_g, half, xt[:, c % 2, lo:lo + 256],
                            c, ncols=256)
                if half == 0:
                    dump = dpool.tile([2, 2048], F32, tag="dump",
                                      name="dump_tail")
                    nc.scalar.activation(
                        dump[:2, :256],
                        ps_g[0:2, 0, 0:256],
                        mybir.ActivationFunctionType.Tanh,
                        bias=aff_t[:, 1:2], scale=aff_t[:, 0:1],
                        accum_out=sums[0:2, 8:9],
                    )
                else:
                    nc.scalar.activation(
                        sums[0:2, N_GROUPS:SUMW],
                        ps_g[0:2, 1, 0:256],
                        mybir.ActivationFunctionType.Tanh,
                        bias=aff_t[:, 1:2], scale=aff_t[:, 0:1],
                    )

            nc.sync.dma_start(out=out[:], in_=sums[:])

    nc.compile()
    return nc


def _get_nc():
    global _nc_cache
    if _nc_cache is None:
        _nc_cache = _build_nc()
    return _nc_cache


def _get_exec():
    """Compile the 8-core SPMD executable once and cache the jitted callable
    (run_bass_kernel_spmd rebuilds + recompiles the NEFF on every call)."""
    global _exec_cache
    if _exec_cache is not None:
        return _exec_cache

    import jax
    import concourse.mybir as _mybir
    from concourse import bass2jax
    from jax.experimental.shard_map import shard_map
    from jax.sharding import Mesh, PartitionSpec

    nc = _get_nc()
    bass2jax.install_neuronx_cc_hook()

    partition_name = (nc.partition_id_tensor.name
                      if nc.partition_id_tensor else None)
    in_names, out_names, out_avals = [], [], []
    for alloc in nc.m.functions[0].allocations:
        if not isinstance(alloc, _mybir.MemoryLocationSet):
            continue
        name = alloc.memorylocations[0].name
        if alloc.kind == "ExternalInput":
            if name != partition_name:
                in_names.append(name)
        elif alloc.kind == "ExternalOutput":
            shape = tuple(alloc.tensor_shape)
            dtype = _mybir.dt.np(alloc.dtype)
            out_names.append(name)
            out_avals.append(jax.core.ShapedArray(shape, dtype))
    n_params = len(in_names)
    all_in_names = list(in_names) + list(out_names)
    if partition_name is not None:
        all_in_names.append(partition_name)

    def _body(*args):
        operands = list(args)
        if partition_name is not None:
            operands.append(bass2jax.partition_id_tensor())
        outs = bass2jax._bass_exec_p.bind(
            *operands,
            out_avals=tuple(out_avals),
            in_names=tuple(all_in_names),
            out_names=tuple(out_names),
            lowering_input_output_aliases=(),
            sim_require_finite=True,
            sim_require_nnan=True,
            nc=nc,
        )
        return tuple(outs)

    devices = jax.devices()[:N_CORES]
    mesh = Mesh(np.asarray(devices), ("core",))
    n_outs = len(out_names)
    sharded = jax.jit(
        shard_map(
            _body, mesh=mesh,
            in_specs=(PartitionSpec("core"),) * (n_params + n_outs),
            out_specs=(PartitionSpec("core"),) * n_outs,
            check_rep=False,
        ),
        donate_argnums=tuple(range(n_params, n_params + n_outs)),
        keep_unused=True,
    )
    _exec_cache = (sharded, in_names, out_names, out_avals)
    return _exec_cache


def _run_spmd(in_maps):
    """Run the cached executable; returns list of per-core output dicts."""
    sharded, in_names, out_names, out_avals = _get_exec()
    concat_in = [
        np.concatenate([np.asarray(m[name]) for m in in_maps], axis=0)
        for name in in_names
    ]
    concat_zeros = [
        np.zeros((N_CORES * av.shape[0], *av.shape[1:]), av.dtype)
        for av in out_avals
    ]
    out_arrs = sharded(*concat_in, *concat_zeros)
    return [
        {name: np.asarray(out_arrs[i]).reshape(N_CORES, *out_avals[i].shape)[c]
         for i, name in enumerate(out_names)}
        for c in range(N_CORES)
    ]


def kernel(x: np.ndarray, w: np.ndarray, b: np.ndarray, mode) -> np.ndarray:
    x = np.ascontiguousarray(np.asarray(x, dtype=np.float32))
    w = np.ascontiguousarray(np.asarray(w, dtype=np.float32))
    b = np.ascontiguousarray(np.asarray(b, dtype=np.float32))
    assert x.shape == (B_FULL, C, H, W), x.shape

    b0 = float(b.reshape(-1)[0])
    aff = np.array([[FA, FA * b0 + FB], [0.5, 0.5 * b0]], dtype=np.float32)
    in_maps = [
        {"x": x[i * B_LOCAL:(i + 1) * B_LOCAL], "w": w, "aff": aff}
        for i in range(N_CORES)
    ]
    try:
        results = _run_spmd(in_maps)
    except Exception:
        nc = _get_nc()
        results = run_bass_kernel_spmd(nc, in_maps, list(range(N_CORES))).results
    partial = np.stack([r["out"] for r in results])  # [8, 2, SUMW]

    n_total = float(B_FULL * HW)
    # Row 0/1 mix per-group SUMS and the tail's raw tanh VALUES — both
    # just add.
    sum_f = float(partial[:, 0, :].sum())
    sum_z = float(partial[:, 1, :].sum())
    s_sp = n_total * FC0 + FC1 * sum_f
    s_z = n_total / 2.0 + sum_z / 2.0
    y = float(np.asarray(mode))
    loss = (s_sp - y * s_z) / n_total
    return np.float32(loss)
